# revision 3
# baseline (speedup 1.0000x reference)
"""Trainium2 Bass kernel for DQLinearLoRA (NF4-style blockwise dequant + LoRA linear).

Computes out = x @ dequant(weight).T + (x @ lora_A.T) @ lora_B.T on 8 NeuronCores.

Sharding: tensor-parallel over out_features (each core owns 512 of 4096 rows of
weight / lora_B / max_val blocks); x is replicated. Each core:
  1. dequantizes its weight slice on-chip: 15 fp16 threshold compares against
     codebook midpoints (DVE 4x mode). The 15-mask sum runs on TensorE
     (identity matmuls) for the first NPE k-tiles (filling PE's startup
     window) and as DVE add-chains for the rest, keeping PE for the backbone.
  2. merges the LoRA update (lora_B @ lora_A, computed by TensorE in bf16)
     into the dequantized fp16 weight slab held in SBUF,
  3. streams fp16 x.T tiles through TensorE against the resident slab in
     G=4 contraction groups (8 k-tiles each), accumulating group partials
     into fp16 SBUF accumulators, so the backbone matmul starts as soon as
     the first k-group is dequantized instead of waiting for the full slab.
Host side does layout prep only: transposes, u = w/max normalization (the
same elementwise scaling the device would apply), dtype casts, concat.
"""

import sys
from contextlib import ExitStack

import numpy as np

sys.path.insert(0, "/opt/trn_rl_repo")

import concourse.bacc as bacc
import concourse.mybir as mybir
from concourse import tile
from concourse.bass_utils import run_bass_kernel_spmd

P = 128  # partitions
BLOCK = 64  # quantization block size

# Problem dims (hardcoded per contract)
T_FULL = 8192
IN_F = 4096
OUT_F = 4096
RANK = 64
N_CORES = 8

MODE = "fp16"
NPE = 8  # k-tiles whose mask-sum runs on TensorE (fills PE startup window)
G = 4  # phase-M contraction groups

_CACHE = {}


def _np_dt(dt):
    return np.dtype(mybir.dt.np(dt))


def build_program(T, IF, OPC, R, n_cores, mids, deltas, c0, mode, t_tile=512):
    """Build the per-core SPMD program. mids/deltas/c0: python floats baked in."""
    f32 = mybir.dt.float32
    bf16 = mybir.dt.bfloat16
    f16 = mybir.dt.float16

    KT = IF // P  # k tiles
    OS = OPC // P  # out-feature 128-slices per core
    NTT = T // t_tile  # token tiles
    NLVL = len(mids)  # 15
    KPG = KT // G  # k tiles per phase-M group

    nc = bacc.Bacc(
        "TRN2",
        target_bir_lowering=False,
        debug=False,
        num_devices=n_cores,
    )
    op = mybir.AluOpType

    ident = nc.dram_tensor("ident", [P, P], f16, kind="ExternalInput").ap()
    xT = nc.dram_tensor("xT", [IF, T], f16, kind="ExternalInput").ap()
    uT = nc.dram_tensor("uT", [IF, OPC], f16, kind="ExternalInput").ap()
    maxB = nc.dram_tensor("maxB", [IF, OPC], f16, kind="ExternalInput").ap()
    A = nc.dram_tensor("A", [R, IF], bf16, kind="ExternalInput").ap()
    BT = nc.dram_tensor("BT", [R, OPC], bf16, kind="ExternalInput").ap()
    outT = nc.dram_tensor("outT", [OPC, T], f16, kind="ExternalOutput").ap()

    with tile.TileContext(nc) as tc, ExitStack() as ctx:
        const = ctx.enter_context(tc.tile_pool(name="const", bufs=1))
        A_sb = const.tile([R, IF], bf16)
        nc.sync.dma_start(A_sb[:], A[:])
        BT_sb = const.tile([R, OPC], bf16)
        nc.sync.dma_start(BT_sb[:], BT[:])
        id_sb = const.tile([P, P], f16, name="id_sb")
        nc.sync.dma_start(id_sb[:], ident[:])

        qw_pool = ctx.enter_context(tc.tile_pool(name="qw", bufs=KT))
        wrk = ctx.enter_context(tc.tile_pool(name="wrk", bufs=4))
        msk = ctx.enter_context(tc.tile_pool(name="msk", bufs=8))
        psum = ctx.enter_context(tc.tile_pool(name="psum", bufs=6, space="PSUM"))
        dqps = ctx.enter_context(tc.tile_pool(name="dqps", bufs=2, space="PSUM"))
        bap = ctx.enter_context(tc.tile_pool(name="bap", bufs=KT))

        # ---- Phase L: all LoRA slab tiles first — dense PE work at t=0,
        # evicted to SBUF fp16 so no PSUM bank is held during dequant.
        # (lora_B @ lora_A).T[ksl, :] = A[:, ksl].T @ BT
        ba_tiles = []
        for kt in range(KT):
            ksl = slice(kt * P, (kt + 1) * P)
            ba_ps = psum.tile([P, OPC], f32, tag="ps", name=f"baps{kt}")
            nc.tensor.matmul(ba_ps[:], A_sb[:, ksl], BT_sb[:], start=True, stop=True)
            ba_sb = bap.tile([P, OPC], f16, tag="ba", name=f"ba{kt}")
            nc.scalar.copy(ba_sb[:], ba_ps[:])
            ba_tiles.append(ba_sb)

        # ---- Phase D: dequant weight slice, one [128, OPC] k-tile at a time.
        # u = w/max comes in fp16; the 15-level staircase runs as fp16
        # tensor_scalar compares (DVE 4x perf mode). Mask sum: TensorE
        # identity matmuls for kt < NPE, DVE fp16 add-chain otherwise.
        qw_tiles = []
        for kt in range(KT):
            ksl = slice(kt * P, (kt + 1) * P)
            u_sb = wrk.tile([P, OPC], f16, tag="u")
            nc.sync.dma_start(u_sb[:], uT[ksl, :])
            mx_sb = wrk.tile([P, OPC], f16, tag="mx")
            nc.sync.dma_start(mx_sb[:], maxB[ksl, :])

            qsc = wrk.tile([P, OPC], f16, tag="qsc")
            if kt < NPE:
                dq_ps = dqps.tile([P, OPC], f32, tag="dq", name=f"dq{kt}")
                for j in range(NLVL):
                    tj = msk.tile([P, OPC], f16, tag="tj", name=f"tj{kt}_{j}")
                    nc.vector.tensor_scalar(
                        tj[:], u_sb[:], float(mids[j]), float(deltas[j]),
                        op0=op.is_gt, op1=op.mult,
                    )
                    nc.tensor.matmul(
                        dq_ps[:], id_sb[:], tj[:], start=(j == 0), stop=(j == NLVL - 1)
                    )
                nc.vector.scalar_tensor_tensor(
                    qsc[:], dq_ps[:], float(c0), mx_sb[:], op0=op.add, op1=op.mult
                )
            else:
                tacc = msk.tile([P, OPC], f16, tag="tacc", bufs=2, name=f"ta{kt}")
                nc.vector.tensor_scalar(
                    tacc[:], u_sb[:], float(mids[0]), float(deltas[0]),
                    op0=op.is_gt, op1=op.mult,
                )
                for j in range(1, NLVL):
                    tj = msk.tile([P, OPC], f16, tag="tj", name=f"tj{kt}_{j}")
                    nc.vector.tensor_scalar(
                        tj[:], u_sb[:], float(mids[j]), float(deltas[j]),
                        op0=op.is_gt, op1=op.mult,
                    )
                    nc.vector.tensor_tensor(tacc[:], tacc[:], tj[:], op=op.add)
                nc.vector.scalar_tensor_tensor(
                    qsc[:], tacc[:], float(c0), mx_sb[:], op0=op.add, op1=op.mult
                )
            # qw = qsc + (lora_B@lora_A).T tile
            qw_sb = qw_pool.tile([P, OPC], f16, tag="qwt")
            nc.vector.tensor_tensor(qw_sb[:], qsc[:], ba_tiles[kt][:], op=op.add)
            qw_tiles.append(qw_sb)

        # ---- Phase M: backbone matmul in G contraction groups. Group g's
        # chains need only qw tiles [g*KPG, (g+1)*KPG), so PE starts after the
        # first group is dequantized. Partials accumulate in fp16 SBUF tiles.
        xp = ctx.enter_context(tc.tile_pool(name="xp", bufs=16))
        ob = ctx.enter_context(tc.tile_pool(name="ob", bufs=4))
        accp = ctx.enter_context(tc.tile_pool(name="accp", bufs=NTT * OS))
        acc = {}
        for g in range(G):
            for tt in range(NTT):
                tsl = slice(tt * t_tile, (tt + 1) * t_tile)
                xs = {}
                for kt in range(g * KPG, (g + 1) * KPG):
                    x_sb = xp.tile([P, t_tile], f16, tag="x", name=f"x{tt}_{kt}")
                    nc.sync.dma_start(x_sb[:], xT[kt * P : (kt + 1) * P, tsl])
                    xs[kt] = x_sb
                ps = {}
                for o in range(OS):
                    ps[o] = psum.tile([P, t_tile], f32, tag="ps", name=f"ps{g}_{tt}_{o}")
                    for i, kt in enumerate(range(g * KPG, (g + 1) * KPG)):
                        nc.tensor.matmul(
                            ps[o][:],
                            qw_tiles[kt][:, o * P : (o + 1) * P],
                            xs[kt][:],
                            start=(i == 0),
                            stop=(i == KPG - 1),
                        )
                for o in range(OS):
                    if g == 0:
                        a_sb = accp.tile([P, t_tile], f16, tag="acc", name=f"acc{tt}_{o}")
                        nc.scalar.copy(a_sb[:], ps[o][:])
                        acc[(tt, o)] = a_sb
                    elif g < G - 1:
                        nc.vector.tensor_tensor(
                            acc[(tt, o)][:], ps[o][:], acc[(tt, o)][:], op=op.add
                        )
                    else:
                        o_sb = ob.tile([P, t_tile], f16, tag="osb", name=f"ob{tt}_{o}")
                        nc.vector.tensor_tensor(
                            o_sb[:], ps[o][:], acc[(tt, o)][:], op=op.add
                        )
                        nc.sync.dma_start(outT[o * P : (o + 1) * P, tsl], o_sb[:])

    nc.compile()
    return nc


def _lut_consts(lookup_table):
    lut = np.asarray(lookup_table, np.float64)
    mids = ((lut[:-1] + lut[1:]) / 2).astype(np.float32)
    deltas = (lut[1:] - lut[:-1]).astype(np.float32)
    c0 = np.float32(lut[0])
    return mids, deltas, c0


def prep_inputs(x, weight, lora_A, lora_B, max_val, mode, n_cores=N_CORES):
    """Host-side sharding/layout prep. Returns in_maps (one dict per core)."""
    f32 = np.float32
    f16 = np.float16
    bf16 = _np_dt(mybir.dt.bfloat16)
    T, IF = x.shape
    OF = weight.shape[0]
    OPC = OF // n_cores

    xT = np.ascontiguousarray(np.asarray(x, f32).T).astype(f16)
    A = np.ascontiguousarray(np.asarray(lora_A, f32)).astype(bf16)
    maxR = np.asarray(max_val, f32).reshape(OF, IF // BLOCK)  # [o, block]
    w = np.asarray(weight, f32)
    # u = w / max per 64-block along in_features (same elementwise scaling the
    # device would compute via w * (1/max)); shipped as fp16
    u = w / np.repeat(maxR, BLOCK, axis=1)
    B = np.asarray(lora_B, f32)

    in_maps = []
    for c in range(n_cores):
        osl = slice(c * OPC, (c + 1) * OPC)
        uT_c = np.ascontiguousarray(u[osl].T).astype(f16)  # [IF, OPC]
        mx_c = np.repeat(maxR[osl].T, BLOCK, axis=0).astype(f16)  # [IF, OPC]
        in_maps.append(
            {
                "ident": np.eye(P, dtype=f16),
                "xT": xT,
                "uT": uT_c,
                "maxB": mx_c,
                "A": A,
                "BT": np.ascontiguousarray(B[osl].T).astype(bf16),  # [R, OPC]
            }
        )
    return in_maps


def _get_program(mids, deltas, c0, mode):
    key = (mode, tuple(np.asarray(mids).tolist()), tuple(np.asarray(deltas).tolist()), float(c0))
    if key not in _CACHE:
        _CACHE[key] = build_program(
            T_FULL, IN_F, OUT_F // N_CORES, RANK, N_CORES, mids, deltas, c0, mode
        )
    return _CACHE[key]


def kernel(x, weight, lora_A, lora_B, max_val, lookup_table):
    mids, deltas, c0 = _lut_consts(lookup_table)
    nc = _get_program(mids, deltas, c0, MODE)
    in_maps = prep_inputs(x, weight, lora_A, lora_B, max_val, MODE)
    res = run_bass_kernel_spmd(nc, in_maps, core_ids=list(range(N_CORES))).results
    outT = np.concatenate([res[c]["outT"] for c in range(N_CORES)], axis=0)  # [OF, T]
    return np.ascontiguousarray(outT.T).astype(np.float32)


# revision 4
# speedup vs baseline: 1.1698x; 1.1698x over previous
"""Trainium2 Bass kernel for DQLinearLoRA (NF4-style blockwise dequant + LoRA linear).

Computes out = x @ dequant(weight).T + (x @ lora_A.T) @ lora_B.T on 8 NeuronCores.

Sharding: tensor-parallel over out_features (each core owns 512 of 4096 rows of
weight / lora_B / max_val blocks); x is replicated. Each core:
  1. dequantizes its weight slice on-chip: 15 fp16 threshold compares against
     codebook midpoints on DVE. The 15-mask sum runs on TensorE (identity
     matmuls) for the first NPE k-tiles (filling PE's startup window) and as
     DVE add-chains for the rest, keeping PE for the backbone matmul.
  2. merges the LoRA update (lora_B @ lora_A, computed by TensorE in bf16)
     into the dequantized fp16 weight slab held in SBUF,
  3. streams fp16 x.T tiles through TensorE against the resident slab in
     G=4 contraction groups (8 k-tiles each), accumulating group partials
     into fp16 SBUF accumulators, so the backbone matmul starts as soon as
     the first k-group is dequantized instead of waiting for the full slab.
Emission interleaves late dequant tiles between phase-M chain batches so DVE
serves PSUM evict-adds on schedule (bank-starvation avoidance).
Host side does layout prep only: transposes, u = w/max normalization (the
same elementwise scaling the device would apply), dtype casts, concat.
"""

import sys
from contextlib import ExitStack

import numpy as np

sys.path.insert(0, "/opt/trn_rl_repo")

import concourse.bacc as bacc
import concourse.mybir as mybir
from concourse import tile
from concourse.bass_utils import run_bass_kernel_spmd

P = 128  # partitions
BLOCK = 64  # quantization block size

# Problem dims (hardcoded per contract)
T_FULL = 8192
IN_F = 4096
OUT_F = 4096
RANK = 64
N_CORES = 8

MODE = "fp16"
NPE = 8  # k-tiles whose mask-sum runs on TensorE (fills PE startup window)
G = 4  # phase-M contraction groups

_CACHE = {}


def _np_dt(dt):
    return np.dtype(mybir.dt.np(dt))


def build_program(T, IF, OPC, R, n_cores, mids, deltas, c0, mode, t_tile=512):
    """Build the per-core SPMD program. mids/deltas/c0: python floats baked in."""
    f32 = mybir.dt.float32
    bf16 = mybir.dt.bfloat16
    f16 = mybir.dt.float16

    KT = IF // P  # k tiles
    OS = OPC // P  # out-feature 128-slices per core
    NTT = T // t_tile  # token tiles
    NLVL = len(mids)  # 15
    KPG = KT // G  # k tiles per phase-M group

    nc = bacc.Bacc(
        "TRN2",
        target_bir_lowering=False,
        debug=False,
        num_devices=n_cores,
    )
    op = mybir.AluOpType

    ident = nc.dram_tensor("ident", [P, P], f16, kind="ExternalInput").ap()
    xT = nc.dram_tensor("xT", [IF, T], f16, kind="ExternalInput").ap()
    uT = nc.dram_tensor("uT", [IF, OPC], f16, kind="ExternalInput").ap()
    maxB = nc.dram_tensor("maxB", [IF, OPC], f16, kind="ExternalInput").ap()
    A = nc.dram_tensor("A", [R, IF], bf16, kind="ExternalInput").ap()
    BT = nc.dram_tensor("BT", [R, OPC], bf16, kind="ExternalInput").ap()
    outT = nc.dram_tensor("outT", [OPC, T], f16, kind="ExternalOutput").ap()

    with tile.TileContext(nc) as tc, ExitStack() as ctx:
        const = ctx.enter_context(tc.tile_pool(name="const", bufs=1))
        A_sb = const.tile([R, IF], bf16)
        nc.sync.dma_start(A_sb[:], A[:])
        BT_sb = const.tile([R, OPC], bf16)
        nc.sync.dma_start(BT_sb[:], BT[:])
        id_sb = const.tile([P, P], f16, name="id_sb")
        nc.sync.dma_start(id_sb[:], ident[:])

        qw_pool = ctx.enter_context(tc.tile_pool(name="qw", bufs=KT))
        wrk = ctx.enter_context(tc.tile_pool(name="wrk", bufs=4))
        msk = ctx.enter_context(tc.tile_pool(name="msk", bufs=8))
        psum = ctx.enter_context(tc.tile_pool(name="psum", bufs=6, space="PSUM"))
        dqps = ctx.enter_context(tc.tile_pool(name="dqps", bufs=2, space="PSUM"))
        bap = ctx.enter_context(tc.tile_pool(name="bap", bufs=KT))
        xp = ctx.enter_context(tc.tile_pool(name="xp", bufs=16))
        ob = ctx.enter_context(tc.tile_pool(name="ob", bufs=4))
        accp = ctx.enter_context(tc.tile_pool(name="accp", bufs=NTT * OS))

        # ---- Phase L: all LoRA slab tiles first — dense PE work at t=0,
        # evicted to SBUF fp16 so no PSUM bank is held during dequant.
        # (lora_B @ lora_A).T[ksl, :] = A[:, ksl].T @ BT
        ba_tiles = []
        for kt in range(KT):
            ksl = slice(kt * P, (kt + 1) * P)
            ba_ps = psum.tile([P, OPC], f32, tag="ps", name=f"baps{kt}")
            nc.tensor.matmul(ba_ps[:], A_sb[:, ksl], BT_sb[:], start=True, stop=True)
            ba_sb = bap.tile([P, OPC], f16, tag="ba", name=f"ba{kt}")
            nc.scalar.copy(ba_sb[:], ba_ps[:])
            ba_tiles.append(ba_sb)

        qw_tiles = {}

        def emit_dq(kt):
            # Dequant one [128, OPC] k-tile. u = w/max comes in fp16; the
            # staircase is 15 fp16 tensor_scalar compares on DVE. Sum on
            # TensorE identity matmuls (kt < NPE) or DVE add-chain.
            ksl = slice(kt * P, (kt + 1) * P)
            u_sb = wrk.tile([P, OPC], f16, tag="u", name=f"u{kt}")
            nc.sync.dma_start(u_sb[:], uT[ksl, :])
            mx_sb = wrk.tile([P, OPC], f16, tag="mx", name=f"mx{kt}")
            nc.sync.dma_start(mx_sb[:], maxB[ksl, :])

            qsc = wrk.tile([P, OPC], f16, tag="qsc", name=f"qsc{kt}")
            if kt < NPE:
                dq_ps = dqps.tile([P, OPC], f32, tag="dq", name=f"dq{kt}")
                for j in range(NLVL):
                    tj = msk.tile([P, OPC], f16, tag="tj", name=f"tj{kt}_{j}")
                    nc.vector.tensor_scalar(
                        tj[:], u_sb[:], float(mids[j]), float(deltas[j]),
                        op0=op.is_gt, op1=op.mult,
                    )
                    nc.tensor.matmul(
                        dq_ps[:], id_sb[:], tj[:], start=(j == 0), stop=(j == NLVL - 1)
                    )
                nc.vector.scalar_tensor_tensor(
                    qsc[:], dq_ps[:], float(c0), mx_sb[:], op0=op.add, op1=op.mult
                )
            else:
                tacc = msk.tile([P, OPC], f16, tag="tacc", bufs=2, name=f"ta{kt}")
                nc.vector.tensor_scalar(
                    tacc[:], u_sb[:], float(mids[0]), float(deltas[0]),
                    op0=op.is_gt, op1=op.mult,
                )
                for j in range(1, NLVL):
                    tj = msk.tile([P, OPC], f16, tag="tj", name=f"tj{kt}_{j}")
                    nc.vector.tensor_scalar(
                        tj[:], u_sb[:], float(mids[j]), float(deltas[j]),
                        op0=op.is_gt, op1=op.mult,
                    )
                    nc.vector.tensor_tensor(tacc[:], tacc[:], tj[:], op=op.add)
                nc.vector.scalar_tensor_tensor(
                    qsc[:], tacc[:], float(c0), mx_sb[:], op0=op.add, op1=op.mult
                )
            # qw = qsc + (lora_B@lora_A).T tile
            qw_sb = qw_pool.tile([P, OPC], f16, tag="qwt", name=f"qw{kt}")
            nc.vector.tensor_tensor(qw_sb[:], qsc[:], ba_tiles[kt][:], op=op.add)
            qw_tiles[kt] = qw_sb

        acc = {}

        def emit_m(g, tts):
            # Phase-M chains for contraction group g over token tiles tts.
            for tt in tts:
                tsl = slice(tt * t_tile, (tt + 1) * t_tile)
                xs = {}
                for kt in range(g * KPG, (g + 1) * KPG):
                    x_sb = xp.tile([P, t_tile], f16, tag="x", name=f"x{tt}_{kt}")
                    nc.sync.dma_start(x_sb[:], xT[kt * P : (kt + 1) * P, tsl])
                    xs[kt] = x_sb
                ps = {}
                for o in range(OS):
                    ps[o] = psum.tile([P, t_tile], f32, tag="ps", name=f"ps{g}_{tt}_{o}")
                    for i, kt in enumerate(range(g * KPG, (g + 1) * KPG)):
                        nc.tensor.matmul(
                            ps[o][:],
                            qw_tiles[kt][:, o * P : (o + 1) * P],
                            xs[kt][:],
                            start=(i == 0),
                            stop=(i == KPG - 1),
                        )
                for o in range(OS):
                    if g == 0:
                        a_sb = accp.tile([P, t_tile], f16, tag="acc", name=f"acc{tt}_{o}")
                        nc.scalar.copy(a_sb[:], ps[o][:])
                        acc[(tt, o)] = a_sb
                    elif g < G - 1:
                        nc.vector.tensor_tensor(
                            acc[(tt, o)][:], ps[o][:], acc[(tt, o)][:], op=op.add
                        )
                    else:
                        o_sb = ob.tile([P, t_tile], f16, tag="osb", name=f"ob{tt}_{o}")
                        nc.vector.tensor_tensor(
                            o_sb[:], ps[o][:], acc[(tt, o)][:], op=op.add
                        )
                        nc.sync.dma_start(outT[o * P : (o + 1) * P, tsl], o_sb[:])

        # Deadline-aware interleaving: late dequant tiles are emitted between
        # phase-M batches so DVE alternates dequant with evict-adds.
        H = NTT // 2
        for kt in range(16):
            emit_dq(kt)
        emit_m(0, range(NTT))
        for kt in range(16, 20):
            emit_dq(kt)
        emit_m(1, range(H))
        for kt in range(20, 24):
            emit_dq(kt)
        emit_m(1, range(H, NTT))
        for kt in range(24, 28):
            emit_dq(kt)
        emit_m(2, range(H))
        for kt in range(28, 32):
            emit_dq(kt)
        emit_m(2, range(H, NTT))
        emit_m(3, range(NTT))

    nc.compile()
    return nc


def _lut_consts(lookup_table):
    lut = np.asarray(lookup_table, np.float64)
    mids = ((lut[:-1] + lut[1:]) / 2).astype(np.float32)
    deltas = (lut[1:] - lut[:-1]).astype(np.float32)
    c0 = np.float32(lut[0])
    return mids, deltas, c0


def prep_inputs(x, weight, lora_A, lora_B, max_val, mode, n_cores=N_CORES):
    """Host-side sharding/layout prep. Returns in_maps (one dict per core)."""
    f32 = np.float32
    f16 = np.float16
    bf16 = _np_dt(mybir.dt.bfloat16)
    T, IF = x.shape
    OF = weight.shape[0]
    OPC = OF // n_cores

    xT = np.ascontiguousarray(np.asarray(x, f32).T).astype(f16)
    A = np.ascontiguousarray(np.asarray(lora_A, f32)).astype(bf16)
    maxR = np.asarray(max_val, f32).reshape(OF, IF // BLOCK)  # [o, block]
    w = np.asarray(weight, f32)
    # u = w / max per 64-block along in_features (same elementwise scaling the
    # device would compute via w * (1/max)); shipped as fp16
    u = w / np.repeat(maxR, BLOCK, axis=1)
    B = np.asarray(lora_B, f32)

    in_maps = []
    for c in range(n_cores):
        osl = slice(c * OPC, (c + 1) * OPC)
        uT_c = np.ascontiguousarray(u[osl].T).astype(f16)  # [IF, OPC]
        mx_c = np.repeat(maxR[osl].T, BLOCK, axis=0).astype(f16)  # [IF, OPC]
        in_maps.append(
            {
                "ident": np.eye(P, dtype=f16),
                "xT": xT,
                "uT": uT_c,
                "maxB": mx_c,
                "A": A,
                "BT": np.ascontiguousarray(B[osl].T).astype(bf16),  # [R, OPC]
            }
        )
    return in_maps


def _get_program(mids, deltas, c0, mode):
    key = (mode, tuple(np.asarray(mids).tolist()), tuple(np.asarray(deltas).tolist()), float(c0))
    if key not in _CACHE:
        _CACHE[key] = build_program(
            T_FULL, IN_F, OUT_F // N_CORES, RANK, N_CORES, mids, deltas, c0, mode
        )
    return _CACHE[key]


def kernel(x, weight, lora_A, lora_B, max_val, lookup_table):
    mids, deltas, c0 = _lut_consts(lookup_table)
    nc = _get_program(mids, deltas, c0, MODE)
    in_maps = prep_inputs(x, weight, lora_A, lora_B, max_val, MODE)
    res = run_bass_kernel_spmd(nc, in_maps, core_ids=list(range(N_CORES))).results
    outT = np.concatenate([res[c]["outT"] for c in range(N_CORES)], axis=0)  # [OF, T]
    return np.ascontiguousarray(outT.T).astype(np.float32)
